# revision 18
# baseline (speedup 1.0000x reference)
"""Distributed Bass kernel for nn_Attention (dense transformer block with the
q=k=v source bug) on 8 TRN2 NeuronCores.

Sharding: tensor-parallel over heads (Megatron-style). Core i owns heads
(2i, 2i+1):
  1. k-projection: KT[d, t] = (x @ W_k_slice).T computed from host-transposed
     x^T so no on-device transpose of x is needed.
  2. Scores S = K K^T are SYMMETRIC (q == k), so tiles are produced in
     [key-block, query] layout directly; row sums equal the softmax
     denominators (row sums == column sums).
  3. exp is split across TWO engines: ScalarE (exact LUT exp, accum_out gives
     row sums for free) for 12/16 key blocks, VectorE (Schraudolph bit-trick
     exp in the bf16 domain: bits = round(s*16/ln2 + 16248) viewed as bf16,
     plus an explicit row-sum reduce) for 4/16 key blocks. This keeps the
     scalar engine (the former bottleneck at ~180us) under ~140us.
  4. O^T = K^T @ E accumulated in PSUM (col-packed: 2 heads side by side),
     normalized by 1/s.
  5. Row-parallel output projection: y_i = OT_i^T @ W_proj[128i:128i+128, :]
     for ALL tokens -> each core emits a full-shape PARTIAL [B, L, D] in bf16;
     the host sums the 8 partials + bias. This removes the AllToAll whose
     exposed wait dominated the baseline (~100us).

All matmuls run in bf16; PSUM accumulation is f32.
"""

import numpy as np

import concourse.bass as bass
import concourse.tile as tile
from concourse import bacc, mybir
from concourse.bass_utils import run_bass_kernel_spmd
from concourse.masks import make_identity

N_CORES = 8
B, L, D = 2, 2048, 1024
H, HD = 16, 64
HPC = H // N_CORES  # heads per core = 2
DC = HPC * HD  # head-dim columns per core = 128
F32 = mybir.dt.float32
BF16 = mybir.dt.bfloat16
I16 = mybir.dt.int16
QS = 1024  # query span per S-tile (PSUM budget: 2 banks)
KB = L // 128  # 16 key blocks per batch
DVE_KBS = (2, 5, 8, 11, 14)  # key blocks whose exp runs on VectorE
# Schraudolph exp in bf16 bit domain: exp(0.125*s) ~= bf16_bits(round(s*A + C))
EXP_A = 0.125 * 128.0 / float(np.log(2.0))
EXP_C = 127.0 * 128.0 - 8.0


def build():
    nc = bacc.Bacc("TRN2", target_bir_lowering=False, debug=False, num_devices=N_CORES)
    xt = nc.dram_tensor("xt", [B, D, L], BF16, kind="ExternalInput")
    wk = nc.dram_tensor("wk", [D, DC], BF16, kind="ExternalInput")
    bk = nc.dram_tensor("bk", [DC, 1], F32, kind="ExternalInput")
    wp = nc.dram_tensor("wp", [DC, D], BF16, kind="ExternalInput")
    out = nc.dram_tensor("out", [B, L, D], BF16, kind="ExternalOutput")

    xt_v = xt.ap().rearrange("b (dc p) t -> b dc p t", p=128)  # [B, 8, 128, L]
    wk_v = wk.ap().rearrange("(dc p) m -> p dc m", p=128)  # [128, 8, DC]

    with tile.TileContext(nc) as tc:
        with (
            tc.tile_pool(name="consts", bufs=1) as consts,
            tc.tile_pool(name="big", bufs=1) as big,
            tc.tile_pool(name="xtp", bufs=1) as xtp,
            tc.tile_pool(name="fpool", bufs=6) as fpool,
            tc.tile_pool(name="small", bufs=4) as small,
            tc.tile_pool(name="rrp", bufs=2) as rrp,
            tc.tile_pool(name="yp", bufs=2) as yp,
            tc.tile_pool(name="ps_s", bufs=2, space="PSUM") as ps_s,
            tc.tile_pool(name="ps_ot", bufs=1, space="PSUM") as ps_ot,
            tc.tile_pool(name="ps_k", bufs=2, space="PSUM") as ps_k,
            tc.tile_pool(name="dram", bufs=1, space="DRAM") as dram,
        ):
            # ---- constants ----
            wk_sb = consts.tile([128, 8, DC], BF16)
            nc.sync.dma_start(wk_sb[:], wk_v)
            bk_sb = consts.tile([128, 1], F32)
            nc.sync.dma_start(bk_sb[:], bk.ap())
            wp_sb = consts.tile([128, D], BF16)
            nc.sync.dma_start(wp_sb[:], wp.ap())
            identb = consts.tile([128, 128], BF16)
            make_identity(nc, identb[:])
            identf = consts.tile([128, 128], F32)
            make_identity(nc, identf[:])

            # persistent activations — separate tiles per batch (and per 512-token
            # chunk for KT) so cross-batch writes never alias reads
            KTc = [[big.tile([128, 512], BF16, name=f"kt{b}_{c}") for c in range(4)]
                   for b in range(B)]
            KNs = [big.tile([128, KB, 128], BF16, name=f"kn{b}") for b in range(B)]
            OTns = [big.tile([128, L], BF16, name=f"otn{b}") for b in range(B)]
            saccs = [big.tile([128, HPC, KB, L // QS], F32, name=f"sacc{b}") for b in range(B)]

            rsf_d = dram.tile([B, HPC, KB, 128], BF16)  # 1/s, token-major flat

            XTs = [[xtp.tile([128, 8, 512], BF16, name=f"xts{b}_{c}", tag=f"xt{c}")
                    for c in range(4)] for b in range(B)]

            def load_xt(b):
                # HWDGE (sync/scalar) rings: lower first-byte latency than the
                # gpsimd SWDGE path and keeps the gpsimd queue free
                for c in range(4):
                    for dc in range(8):
                        q = nc.sync if dc % 2 == 0 else nc.scalar
                        q.dma_start(
                            XTs[b][c][:, dc, :], xt_v[b, dc, :, c * 512 : (c + 1) * 512]
                        )

            def kproj(b):
                for ncx in range(L // 512):
                    kp_ps = ps_k.tile([128, 512], F32, tag="k")
                    for dc in range(8):
                        nc.tensor.matmul(
                            kp_ps[:],
                            lhsT=wk_sb[:, dc, :],
                            rhs=XTs[b][ncx][:, dc, :],
                            start=(dc == 0),
                            stop=(dc == 7),
                        )
                    nc.vector.tensor_scalar_add(KTc[b][ncx][:], kp_ps[:], bk_sb[:])
                    for sub in range(4):
                        tcx = ncx * 4 + sub
                        tp_ps = ps_k.tile([128, 512], BF16, tag="k")
                        nc.tensor.transpose(
                            tp_ps[:, :128],
                            KTc[b][ncx][:, sub * 128 : (sub + 1) * 128],
                            identb[:],
                        )
                        nc.vector.tensor_copy(KNs[b][:, tcx, :], tp_ps[:, :128])

            def attention(b):
                for qs_i in range(L // QS):
                    q0 = qs_i * QS
                    ot_ps = ps_ot.tile([128, QS], F32, tag="ot")
                    for h in range(HPC):
                        hp = 64 * h

                        def av(kb, f_t, start, stop):
                            for qc in range(QS // 512):
                                nc.tensor.matmul(
                                    ot_ps[hp : hp + 64, qc * 512 : (qc + 1) * 512],
                                    lhsT=KNs[b][:, kb, hp : hp + 64],
                                    rhs=f_t[:, qc * 512 : (qc + 1) * 512],
                                    start=start,
                                    stop=stop,
                                    tile_position=(0, hp),
                                )

                        # Software-pipelined AV: each key block's AV matmuls are
                        # emitted 1 (ScalarE exp) or 2 (VectorE exp) key blocks
                        # late so the in-order PE queue never waits on the exp
                        # engines at issue time.
                        pending = []  # (kb, f_t, lag)
                        for kb in range(KB):
                            s_ps = ps_s.tile([128, QS], F32, tag="s")
                            for qc in range(QS // 512):
                                nc.tensor.matmul(
                                    s_ps[:, qc * 512 : (qc + 1) * 512],
                                    lhsT=KTc[b][kb // 4][hp : hp + 64, (kb % 4) * 128 : (kb % 4 + 1) * 128],
                                    rhs=KTc[b][qs_i * 2 + qc][hp : hp + 64, :],
                                    start=True,
                                    stop=True,
                                    tile_position=(hp, 0),
                                )
                            f_t = fpool.tile([128, QS], BF16, tag="f")
                            if kb in DVE_KBS:
                                # Schraudolph exp on VectorE: int16 bits viewed as bf16
                                nc.vector.tensor_scalar(
                                    f_t[:].bitcast(I16),
                                    s_ps[:],
                                    EXP_A,
                                    EXP_C,
                                    mybir.AluOpType.mult,
                                    mybir.AluOpType.add,
                                )
                                nc.vector.tensor_reduce(
                                    saccs[b][:, h, kb, qs_i : qs_i + 1],
                                    f_t[:],
                                    mybir.AxisListType.X,
                                    mybir.AluOpType.add,
                                )
                                pending.append((kb, f_t, 2))
                            else:
                                nc.scalar.activation(
                                    f_t[:],
                                    s_ps[:],
                                    mybir.ActivationFunctionType.Exp,
                                    scale=0.125,
                                    accum_out=saccs[b][:, h, kb, qs_i : qs_i + 1],
                                )
                                pending.append((kb, f_t, 1))
                            while pending and pending[0][0] <= kb - pending[0][2]:
                                pkb, pf, _ = pending.pop(0)
                                av(pkb, pf, start=(pkb == 0), stop=False)
                        for di, (pkb, pf, _) in enumerate(pending):
                            av(pkb, pf, start=(pkb == 0), stop=(di == len(pending) - 1))
                    nc.vector.tensor_copy(OTns[b][:, q0 : q0 + QS], ot_ps[:])

            def normalize(b):
                # 1/s -> token-major DRAM -> partition-broadcast into rr;
                # per-head so h0's chain hides under h1's exps
                rr = rrp.tile([128, L], BF16, tag="rr")
                for h in range(HPC):
                    s_t = small.tile([128, KB], F32, tag="s1")
                    nc.vector.tensor_add(
                        s_t[:], saccs[b][:, h, :, 0], saccs[b][:, h, :, 1]
                    )
                    rs_t = small.tile([128, KB], F32, tag="s2")
                    nc.vector.reciprocal(rs_t[:], s_t[:])
                    rst_ps = ps_k.tile([128, 512], F32, tag="k")
                    nc.tensor.transpose(rst_ps[:KB, :128], rs_t[:], identf[:])
                    rsT = small.tile([KB, 128], BF16, tag="s3")
                    nc.vector.tensor_copy(rsT[:], rst_ps[:KB, :128])
                    nc.sync.dma_start(rsf_d[b, h, :, :], rsT[:])
                    nc.sync.dma_start(
                        rr[64 * h : 64 * (h + 1), :],
                        rsf_d[b, h, :, :].rearrange("a c -> (a c)").partition_broadcast(64),
                    )
                    nc.vector.tensor_mul(
                        OTns[b][64 * h : 64 * (h + 1), :],
                        OTns[b][64 * h : 64 * (h + 1), :],
                        rr[64 * h : 64 * (h + 1), :],
                    )

            def proj(b):
                # row-parallel partial projection: y[t, :] += OTn[:, t]^T W_proj-rows
                # Batch 0 overlaps attention(1): keep ScalarE free for exp, so
                # evacuation runs on VectorE only. Batch 1 is the tail (nothing
                # else running): alternate VectorE / ScalarE to drain in parallel.
                # y tiles hold two 128-token chunks -> half as many output DMAs.
                for tp in range(L // 256):
                    y_t = yp.tile([128, 2, D], BF16, tag="y")
                    for half in range(2):
                        tcx = tp * 2 + half
                        for nc2 in range(D // 512):
                            pj_ps = ps_k.tile([128, 512], F32, tag="k")
                            nc.tensor.matmul(
                                pj_ps[:],
                                lhsT=OTns[b][:, tcx * 128 : (tcx + 1) * 128],
                                rhs=wp_sb[:, nc2 * 512 : (nc2 + 1) * 512],
                                start=True,
                                stop=True,
                            )
                            if b == 1 and (tcx * 2 + nc2) % 2 == 1:
                                nc.scalar.copy(
                                    y_t[:, half, nc2 * 512 : (nc2 + 1) * 512], pj_ps[:]
                                )
                            else:
                                nc.vector.tensor_copy(
                                    y_t[:, half, nc2 * 512 : (nc2 + 1) * 512], pj_ps[:]
                                )
                    q = (nc.sync, nc.gpsimd, nc.scalar)[tp % 3]
                    q.dma_start(
                        out.ap()[b, tp * 256 : (tp + 1) * 256, :].rearrange(
                            "(two p) d -> p two d", p=128
                        ),
                        y_t[:],
                    )

            # ---- schedule (program order = scheduler priority) ----
            # kproj(1) sits AFTER attention(0): its matmuls fill attention(0)'s
            # PE stall gaps instead of delaying the first scores
            load_xt(0)
            kproj(0)
            load_xt(1)
            attention(0)
            kproj(1)
            attention(1)
            normalize(0)
            proj(0)
            normalize(1)
            proj(1)

    nc.compile()
    return nc


_CACHED = None


def _get_nc():
    global _CACHED
    if _CACHED is None:
        _CACHED = build()
    return _CACHED


def run(inputs, trace=False):
    import ml_dtypes

    bf16 = ml_dtypes.bfloat16
    x = np.asarray(inputs["x"], np.float32)
    W_attn = np.asarray(inputs["W_attn"], np.float32)
    b_attn = np.asarray(inputs["b_attn"], np.float32)
    W_proj = np.asarray(inputs["W_proj"], np.float32)
    b_proj = np.asarray(inputs["b_proj"], np.float32)

    xt = np.ascontiguousarray(x.transpose(0, 2, 1)).astype(bf16)  # [B, D, L]
    in_maps = []
    for i in range(N_CORES):
        c0 = D + i * DC
        in_maps.append(
            {
                "xt": xt,
                "wk": np.ascontiguousarray(W_attn[:, c0 : c0 + DC]).astype(bf16),
                "bk": np.ascontiguousarray(b_attn[c0 : c0 + DC].reshape(DC, 1)),
                "wp": np.ascontiguousarray(W_proj[i * DC : (i + 1) * DC, :]).astype(bf16),
            }
        )

    nc = _get_nc()
    res = run_bass_kernel_spmd(
        nc, in_maps, core_ids=list(range(N_CORES)), trace=trace
    )
    y = np.zeros((B, L, D), np.float32)
    for i in range(N_CORES):
        y += res.results[i]["out"].astype(np.float32)
    y += b_proj
    return y, res


def kernel(**inputs) -> np.ndarray:
    y, _ = run(inputs)
    return y


# revision 21
# speedup vs baseline: 1.1299x; 1.1299x over previous
"""Distributed Bass kernel for nn_Attention (dense transformer block with the
q=k=v source bug) on 8 TRN2 NeuronCores.

Sharding: tensor-parallel over heads (Megatron-style). Core i owns heads
(2i, 2i+1):
  1. k-projection: KT[d, t] = (x @ W_k_slice).T computed from host-transposed
     x^T so no on-device transpose of x is needed.
  2. Scores S = K K^T are SYMMETRIC (q == k), so tiles are produced in
     [key-block, query] layout directly; row sums equal the softmax
     denominators (row sums == column sums).
  3. exp is split across TWO engines: ScalarE (exact LUT exp, accum_out gives
     row sums for free) for 12/16 key blocks, VectorE (Schraudolph bit-trick
     exp in the bf16 domain: bits = round(s*16/ln2 + 16248) viewed as bf16,
     plus an explicit row-sum reduce) for 4/16 key blocks. This keeps the
     scalar engine (the former bottleneck at ~180us) under ~140us.
  4. O^T = K^T @ E accumulated in PSUM (col-packed: 2 heads side by side),
     normalized by 1/s.
  5. Row-parallel output projection: y_i = OT_i^T @ W_proj[128i:128i+128, :]
     for ALL tokens -> each core emits a full-shape PARTIAL [B, L, D] in bf16;
     the host sums the 8 partials + bias. This removes the AllToAll whose
     exposed wait dominated the baseline (~100us).

All matmuls run in bf16; PSUM accumulation is f32.
"""

import numpy as np

import concourse.bass as bass
import concourse.tile as tile
from concourse import bacc, mybir
from concourse.bass_utils import run_bass_kernel_spmd
from concourse.masks import make_identity

N_CORES = 8
B, L, D = 2, 2048, 1024
H, HD = 16, 64
HPC = H // N_CORES  # heads per core = 2
DC = HPC * HD  # head-dim columns per core = 128
F32 = mybir.dt.float32
BF16 = mybir.dt.bfloat16
I16 = mybir.dt.int16
QS = 1024  # query span per S-tile (PSUM budget: 2 banks)
KB = L // 128  # 16 key blocks per batch
DVE_KBS = (3, 7, 11, 15)  # key blocks whose exp runs on VectorE
# Schraudolph exp in bf16 bit domain: exp(0.125*s) ~= bf16_bits(round(s*A + C))
EXP_A = 0.125 * 128.0 / float(np.log(2.0))
EXP_C = 127.0 * 128.0 - 8.0


def build():
    nc = bacc.Bacc("TRN2", target_bir_lowering=False, debug=False, num_devices=N_CORES)
    xt = nc.dram_tensor("xt", [B, D, L], BF16, kind="ExternalInput")
    wk = nc.dram_tensor("wk", [D, DC], BF16, kind="ExternalInput")
    bk = nc.dram_tensor("bk", [DC, 1], F32, kind="ExternalInput")
    wp = nc.dram_tensor("wp", [DC, D], BF16, kind="ExternalInput")
    out = nc.dram_tensor("out", [B, L, D], BF16, kind="ExternalOutput")

    xt_v = xt.ap().rearrange("b (dc p) t -> b dc p t", p=128)  # [B, 8, 128, L]
    wk_v = wk.ap().rearrange("(dc p) m -> p dc m", p=128)  # [128, 8, DC]

    with tile.TileContext(nc) as tc:
        with (
            tc.tile_pool(name="consts", bufs=1) as consts,
            tc.tile_pool(name="big", bufs=1) as big,
            tc.tile_pool(name="xtp", bufs=1) as xtp,
            tc.tile_pool(name="fpool", bufs=6) as fpool,
            tc.tile_pool(name="small", bufs=4) as small,
            tc.tile_pool(name="rrp", bufs=2) as rrp,
            tc.tile_pool(name="yp", bufs=2) as yp,
            tc.tile_pool(name="ps_s", bufs=2, space="PSUM") as ps_s,
            tc.tile_pool(name="ps_ot", bufs=1, space="PSUM") as ps_ot,
            tc.tile_pool(name="ps_k", bufs=2, space="PSUM") as ps_k,
            tc.tile_pool(name="dram", bufs=1, space="DRAM") as dram,
        ):
            # ---- constants ----
            wk_sb = consts.tile([128, 8, DC], BF16)
            nc.sync.dma_start(wk_sb[:], wk_v)
            bk_sb = consts.tile([128, 1], F32)
            nc.sync.dma_start(bk_sb[:], bk.ap())
            wp_sb = consts.tile([128, D], BF16)
            # gpsimd ring: wp isn't needed until proj, keep it off the sync
            # queue that feeds the startup-critical xt loads
            nc.gpsimd.dma_start(wp_sb[:], wp.ap())
            identb = consts.tile([128, 128], BF16)
            make_identity(nc, identb[:])
            identf = consts.tile([128, 128], F32)
            make_identity(nc, identf[:])

            # persistent activations — separate tiles per batch (and per 512-token
            # chunk for KT) so cross-batch writes never alias reads
            KTc = [[big.tile([128, 512], BF16, name=f"kt{b}_{c}") for c in range(4)]
                   for b in range(B)]
            KNs = [big.tile([128, KB, 128], BF16, name=f"kn{b}") for b in range(B)]
            OTns = [big.tile([128, L], BF16, name=f"otn{b}") for b in range(B)]
            saccs = [big.tile([128, HPC, KB, L // QS], F32, name=f"sacc{b}") for b in range(B)]

            rsf_d = dram.tile([B, HPC, KB, 128], BF16)  # 1/s, token-major flat

            XTs = [[xtp.tile([128, 8, 512], BF16, name=f"xts{b}_{c}", tag=f"xt{c}")
                    for c in range(4)] for b in range(B)]

            def load_xt(b):
                # HWDGE (sync/scalar) rings: lower first-byte latency than the
                # gpsimd SWDGE path and keeps the gpsimd queue free
                for c in range(4):
                    for dc in range(8):
                        q = nc.sync if dc % 2 == 0 else nc.scalar
                        q.dma_start(
                            XTs[b][c][:, dc, :], xt_v[b, dc, :, c * 512 : (c + 1) * 512]
                        )

            def kproj(b):
                for ncx in range(L // 512):
                    kp_ps = ps_k.tile([128, 512], F32, tag="k")
                    for dc in range(8):
                        nc.tensor.matmul(
                            kp_ps[:],
                            lhsT=wk_sb[:, dc, :],
                            rhs=XTs[b][ncx][:, dc, :],
                            start=(dc == 0),
                            stop=(dc == 7),
                        )
                    nc.vector.tensor_scalar_add(KTc[b][ncx][:], kp_ps[:], bk_sb[:])
                    for sub in range(4):
                        tcx = ncx * 4 + sub
                        tp_ps = ps_k.tile([128, 512], BF16, tag="k")
                        nc.tensor.transpose(
                            tp_ps[:, :128],
                            KTc[b][ncx][:, sub * 128 : (sub + 1) * 128],
                            identb[:],
                        )
                        nc.vector.tensor_copy(KNs[b][:, tcx, :], tp_ps[:, :128])

            def attention(b):
                for qs_i in range(L // QS):
                    q0 = qs_i * QS
                    ot_ps = ps_ot.tile([128, QS], F32, tag="ot")
                    for h in range(HPC):
                        hp = 64 * h

                        def av(kb, f_t, start, stop):
                            for qc in range(QS // 512):
                                nc.tensor.matmul(
                                    ot_ps[hp : hp + 64, qc * 512 : (qc + 1) * 512],
                                    lhsT=KNs[b][:, kb, hp : hp + 64],
                                    rhs=f_t[:, qc * 512 : (qc + 1) * 512],
                                    start=start,
                                    stop=stop,
                                    tile_position=(0, hp),
                                )

                        # Software-pipelined AV: each key block's AV matmuls are
                        # emitted 1 (ScalarE exp) or 2 (VectorE exp) key blocks
                        # late so the in-order PE queue never waits on the exp
                        # engines at issue time.
                        pending = []  # (kb, f_t, lag)
                        for kb in range(KB):
                            s_ps = ps_s.tile([128, QS], F32, tag="s")
                            for qc in range(QS // 512):
                                nc.tensor.matmul(
                                    s_ps[:, qc * 512 : (qc + 1) * 512],
                                    lhsT=KTc[b][kb // 4][hp : hp + 64, (kb % 4) * 128 : (kb % 4 + 1) * 128],
                                    rhs=KTc[b][qs_i * 2 + qc][hp : hp + 64, :],
                                    start=True,
                                    stop=True,
                                    tile_position=(hp, 0),
                                )
                            f_t = fpool.tile([128, QS], BF16, tag="f")
                            if kb in DVE_KBS:
                                # Schraudolph exp on VectorE: int16 bits viewed as bf16
                                nc.vector.tensor_scalar(
                                    f_t[:].bitcast(I16),
                                    s_ps[:],
                                    EXP_A,
                                    EXP_C,
                                    mybir.AluOpType.mult,
                                    mybir.AluOpType.add,
                                )
                                nc.vector.tensor_reduce(
                                    saccs[b][:, h, kb, qs_i : qs_i + 1],
                                    f_t[:],
                                    mybir.AxisListType.X,
                                    mybir.AluOpType.add,
                                )
                                pending.append((kb, f_t, 2))
                            else:
                                nc.scalar.activation(
                                    f_t[:],
                                    s_ps[:],
                                    mybir.ActivationFunctionType.Exp,
                                    scale=0.125,
                                    accum_out=saccs[b][:, h, kb, qs_i : qs_i + 1],
                                )
                                pending.append((kb, f_t, 1))
                            while pending and pending[0][0] <= kb - pending[0][2]:
                                pkb, pf, _ = pending.pop(0)
                                av(pkb, pf, start=(pkb == 0), stop=False)
                        for di, (pkb, pf, _) in enumerate(pending):
                            av(pkb, pf, start=(pkb == 0), stop=(di == len(pending) - 1))
                    nc.vector.tensor_copy(OTns[b][:, q0 : q0 + QS], ot_ps[:])

            def normalize(b):
                # 1/s -> token-major DRAM -> partition-broadcast into rr;
                # per-head so h0's chain hides under h1's exps
                rr = rrp.tile([128, L], BF16, tag="rr")
                for h in range(HPC):
                    s_t = small.tile([128, KB], F32, tag="s1")
                    nc.vector.tensor_add(
                        s_t[:], saccs[b][:, h, :, 0], saccs[b][:, h, :, 1]
                    )
                    rs_t = small.tile([128, KB], F32, tag="s2")
                    nc.vector.reciprocal(rs_t[:], s_t[:])
                    rst_ps = ps_k.tile([128, 512], F32, tag="k")
                    nc.tensor.transpose(rst_ps[:KB, :128], rs_t[:], identf[:])
                    rsT = small.tile([KB, 128], BF16, tag="s3")
                    nc.vector.tensor_copy(rsT[:], rst_ps[:KB, :128])
                    nc.sync.dma_start(rsf_d[b, h, :, :], rsT[:])
                    nc.sync.dma_start(
                        rr[64 * h : 64 * (h + 1), :],
                        rsf_d[b, h, :, :].rearrange("a c -> (a c)").partition_broadcast(64),
                    )
                    nc.vector.tensor_mul(
                        OTns[b][64 * h : 64 * (h + 1), :],
                        OTns[b][64 * h : 64 * (h + 1), :],
                        rr[64 * h : 64 * (h + 1), :],
                    )

            def proj(b):
                # row-parallel partial projection: y[t, :] += OTn[:, t]^T W_proj-rows
                # Batch 0 overlaps attention(1): keep ScalarE free for exp, so
                # evacuation runs on VectorE only. Batch 1 is the tail (nothing
                # else running): alternate VectorE / ScalarE to drain in parallel.
                # y tiles hold two 128-token chunks -> half as many output DMAs.
                for tp in range(L // 256):
                    y_t = yp.tile([128, 2, D], BF16, tag="y")
                    for half in range(2):
                        tcx = tp * 2 + half
                        for nc2 in range(D // 512):
                            pj_ps = ps_k.tile([128, 512], F32, tag="k")
                            nc.tensor.matmul(
                                pj_ps[:],
                                lhsT=OTns[b][:, tcx * 128 : (tcx + 1) * 128],
                                rhs=wp_sb[:, nc2 * 512 : (nc2 + 1) * 512],
                                start=True,
                                stop=True,
                            )
                            if (tcx * 2 + nc2) % 2 == 1:
                                nc.scalar.copy(
                                    y_t[:, half, nc2 * 512 : (nc2 + 1) * 512], pj_ps[:]
                                )
                            else:
                                nc.vector.tensor_copy(
                                    y_t[:, half, nc2 * 512 : (nc2 + 1) * 512], pj_ps[:]
                                )
                    q = (nc.sync, nc.gpsimd, nc.scalar)[tp % 3]
                    q.dma_start(
                        out.ap()[b, tp * 256 : (tp + 1) * 256, :].rearrange(
                            "(two p) d -> p two d", p=128
                        ),
                        y_t[:],
                    )

            # ---- schedule (program order = scheduler priority) ----
            # kproj(1) sits AFTER attention(0): its matmuls fill attention(0)'s
            # PE stall gaps instead of delaying the first scores
            load_xt(0)
            kproj(0)
            load_xt(1)
            attention(0)
            kproj(1)
            attention(1)
            normalize(0)
            proj(0)
            normalize(1)
            proj(1)

    nc.compile()
    return nc


_CACHED = None


def _get_nc():
    global _CACHED
    if _CACHED is None:
        _CACHED = build()
    return _CACHED


def run(inputs, trace=False):
    import ml_dtypes

    bf16 = ml_dtypes.bfloat16
    x = np.asarray(inputs["x"], np.float32)
    W_attn = np.asarray(inputs["W_attn"], np.float32)
    b_attn = np.asarray(inputs["b_attn"], np.float32)
    W_proj = np.asarray(inputs["W_proj"], np.float32)
    b_proj = np.asarray(inputs["b_proj"], np.float32)

    xt = np.ascontiguousarray(x.transpose(0, 2, 1)).astype(bf16)  # [B, D, L]
    in_maps = []
    for i in range(N_CORES):
        c0 = D + i * DC
        in_maps.append(
            {
                "xt": xt,
                "wk": np.ascontiguousarray(W_attn[:, c0 : c0 + DC]).astype(bf16),
                "bk": np.ascontiguousarray(b_attn[c0 : c0 + DC].reshape(DC, 1)),
                "wp": np.ascontiguousarray(W_proj[i * DC : (i + 1) * DC, :]).astype(bf16),
            }
        )

    nc = _get_nc()
    res = run_bass_kernel_spmd(
        nc, in_maps, core_ids=list(range(N_CORES)), trace=trace
    )
    y = np.zeros((B, L, D), np.float32)
    for i in range(N_CORES):
        y += res.results[i]["out"].astype(np.float32)
    y += b_proj
    return y, res


def kernel(**inputs) -> np.ndarray:
    y, _ = run(inputs)
    return y
